# revision 1
# baseline (speedup 1.0000x reference)
"""Trainium2 Bass kernel for the HOI relation model.

Pipeline per core (2 images each, 8 cores data-parallel over batch):
  1. ROI mean pooling: pooled[d,c] = (1/area_d) * sum_hw mask[d,hw] * F[hw,c]
     computed as 32 K-chunk matmuls (mask stationary [128,32], features
     moving [128,768] in two N=384 halves), bf16 operands, f32 PSUM.
  2. PE-transpose pooled [32,768] -> pooledT [768,32] (6 transposes).
  3. Layer 1 factorized: relu(pair(h,o) @ w1 + b1) = relu(A(h) + B(o) + b1)
     where A = w1[:768].T @ h, B = w1[768:].T @ o  -- the 8x24 pair
     expansion happens AFTER the matmul (broadcast add on DVE).
  4. Layers 2, 3 as plain matmuls on the 384 pair rows (transposed layout).

Host does only O(B*D) prep: box->mask rasterization, score argsort
(baked into mask column order), 1/area, dtype casts, shard/gather.
"""

import numpy as np
import ml_dtypes

import concourse.bass as bass
import concourse.mybir as mybir
import concourse.tile as tile
from concourse import bacc
from concourse.bass_utils import run_bass_kernel_spmd
from concourse.masks import make_identity

N_CORES = 8
B, D, C = 16, 32, 768
NH, NO = 8, 24
NPAIR = NH * NO              # 192 pairs per image
GRID = 64                    # feature grid (896 / 14)
KPIX = GRID * GRID           # 4096 pixels per image
BL = B // N_CORES            # 2 images per core
KCH = KPIX // 128            # 32 K-chunks per image
CG = 4                       # K-chunks per DMA tile
H1, H2, H3 = 512, 256, 117
M = BL * NPAIR               # 384 pair rows per core

F32 = mybir.dt.float32
BF16 = mybir.dt.bfloat16
BF = ml_dtypes.bfloat16

_PROGRAM = None


def _build_program():
    nc = bacc.Bacc("TRN2", target_bir_lowering=False, debug=False,
                   num_devices=N_CORES)
    feat = nc.declare_dram_parameter("feat", [BL, KPIX, C], BF16, isOutput=False)
    maskT = nc.declare_dram_parameter("maskT", [BL, KPIX, D], BF16, isOutput=False)
    inva = nc.declare_dram_parameter("inva", [BL, D], F32, isOutput=False)
    w1 = nc.declare_dram_parameter("w1", [2 * C, H1], BF16, isOutput=False)
    b1 = nc.declare_dram_parameter("b1", [H1], F32, isOutput=False)
    w2 = nc.declare_dram_parameter("w2", [H1, H2], BF16, isOutput=False)
    b2 = nc.declare_dram_parameter("b2", [H2], F32, isOutput=False)
    w3 = nc.declare_dram_parameter("w3", [H2, H3], BF16, isOutput=False)
    b3 = nc.declare_dram_parameter("b3", [H3], F32, isOutput=False)
    out = nc.declare_dram_parameter("out", [M, H3], F32, isOutput=True)

    add = mybir.AluOpType.add
    amax = mybir.AluOpType.max

    with tile.TileContext(nc) as tc:
        with (
            tc.tile_pool(name="singles", bufs=1) as singles,
            tc.tile_pool(name="featp", bufs=6) as featp,
            tc.tile_pool(name="maskp", bufs=6) as maskp,
            tc.tile_pool(name="work", bufs=1) as work,
            tc.tile_pool(name="tmp", bufs=3) as tmpp,
            tc.tile_pool(name="pps", bufs=1, space="PSUM") as pps,
            tc.tile_pool(name="mps", bufs=4, space="PSUM") as mps,
        ):
            # ---- one-time constant loads ----
            ident = singles.tile([32, 32], BF16, tag="ident")
            make_identity(nc, ident)
            w1_sb = singles.tile([128, 12, H1], BF16, tag="w1")
            nc.sync.dma_start(out=w1_sb, in_=w1[:, :].rearrange("(kc p) n -> p kc n", p=128))
            w2_sb = singles.tile([128, 4, H2], BF16, tag="w2")
            nc.sync.dma_start(out=w2_sb, in_=w2[:, :].rearrange("(kc p) n -> p kc n", p=128))
            w3_sb = singles.tile([128, 2, H3], BF16, tag="w3")
            nc.sync.dma_start(out=w3_sb, in_=w3[:, :].rearrange("(kc p) n -> p kc n", p=128))
            b1_sb = singles.tile([128, 4], F32, tag="b1")
            nc.sync.dma_start(out=b1_sb, in_=b1[:].rearrange("(mc p) -> p mc", p=128))
            b2_sb = singles.tile([128, 2], F32, tag="b2")
            nc.sync.dma_start(out=b2_sb, in_=b2[:].rearrange("(mc p) -> p mc", p=128))
            b3_sb = singles.tile([128, H3], F32, tag="b3")
            b3_bcast = bass.AP(tensor=b3[:].tensor, offset=b3[:].offset,
                               ap=[[0, 128], [1, H3]])
            nc.sync.dma_start(out=b3_sb, in_=b3_bcast)
            inva_sb = singles.tile([D, BL], F32, tag="inva")
            nc.sync.dma_start(out=inva_sb, in_=inva[:, :].rearrange("b d -> d b"))

            # persistent activations
            pooledT = work.tile([128, BL, 6, D], BF16, tag="pooledT")
            x1T = work.tile([128, 4, M], BF16, tag="x1T")
            x2T = work.tile([128, 2, M], BF16, tag="x2T")

            # ---- pooling + transpose per image ----
            for img in range(BL):
                ps_a = pps.tile([D, 384], F32, tag=f"pp{img}a")
                ps_b = pps.tile([D, 384], F32, tag=f"pp{img}b")
                for g in range(KCH // CG):
                    f_sb = featp.tile([128, CG, C], BF16, tag="f")
                    nc.sync.dma_start(
                        out=f_sb,
                        in_=feat[img, g * CG * 128:(g + 1) * CG * 128, :]
                        .rearrange("(gc p) c -> p gc c", p=128))
                    m_sb = maskp.tile([128, CG, D], BF16, tag="m")
                    nc.sync.dma_start(
                        out=m_sb,
                        in_=maskT[img, g * CG * 128:(g + 1) * CG * 128, :]
                        .rearrange("(gc p) d -> p gc d", p=128))
                    for gc in range(CG):
                        kk = g * CG + gc
                        nc.tensor.matmul(ps_a, m_sb[:, gc, :], f_sb[:, gc, 0:384],
                                         start=(kk == 0), stop=(kk == KCH - 1))
                        nc.tensor.matmul(ps_b, m_sb[:, gc, :], f_sb[:, gc, 384:768],
                                         start=(kk == 0), stop=(kk == KCH - 1))
                # scale by 1/area, cast to bf16
                pooled = tmpp.tile([D, C], BF16, tag="pooled")
                nc.vector.tensor_scalar_mul(pooled[:, 0:384], ps_a, inva_sb[:, img:img + 1])
                nc.vector.tensor_scalar_mul(pooled[:, 384:768], ps_b, inva_sb[:, img:img + 1])
                # transpose to [C, D] in 6 chunks of 128 channels
                for cc in range(6):
                    ps_t = mps.tile([128, D], BF16, tag="mm")
                    nc.tensor.transpose(ps_t, pooled[:, cc * 128:(cc + 1) * 128], ident)
                    nc.vector.tensor_copy(pooledT[:, img, cc, :], ps_t)

            # ---- layer 1 (factorized over pairs) ----
            for mc in range(4):
                ps_ab = mps.tile([128, BL, D], F32, tag="mm")
                for kc in range(6):
                    nc.tensor.matmul(ps_ab[:, :, 0:NH],
                                     w1_sb[:, kc, mc * 128:(mc + 1) * 128],
                                     pooledT[:, :, kc, 0:NH],
                                     start=(kc == 0), stop=(kc == 5))
                for kc in range(6):
                    nc.tensor.matmul(ps_ab[:, :, NH:D],
                                     w1_sb[:, 6 + kc, mc * 128:(mc + 1) * 128],
                                     pooledT[:, :, kc, NH:D],
                                     start=(kc == 0), stop=(kc == 5))
                ab_sb = tmpp.tile([128, BL, D], F32, tag="ab")
                nc.vector.tensor_copy(ab_sb, ps_ab)
                for img in range(BL):
                    pre = tmpp.tile([128, NH, NO], F32, tag="pre")
                    a_bc = ab_sb[:, img, 0:NH][:, :, None].broadcast_to([128, NH, NO])
                    b_bc = ab_sb[:, img, NH:D][:, None, :].broadcast_to([128, NH, NO])
                    # pre = (A + b1) + B
                    nc.vector.scalar_tensor_tensor(pre, a_bc, b1_sb[:, mc:mc + 1],
                                                   b_bc, op0=add, op1=add)
                    dst = x1T[:, mc, img * NPAIR:(img + 1) * NPAIR] \
                        .rearrange("p (i j) -> p i j", i=NH)
                    nc.vector.tensor_scalar_max(dst, pre, 0.0)

            # ---- layer 2 ----
            for m2 in range(2):
                ps2 = mps.tile([128, M], F32, tag="mm")
                for kc in range(4):
                    nc.tensor.matmul(ps2, w2_sb[:, kc, m2 * 128:(m2 + 1) * 128],
                                     x1T[:, kc, :], start=(kc == 0), stop=(kc == 3))
                nc.vector.tensor_scalar(x2T[:, m2, :], ps2, b2_sb[:, m2:m2 + 1], 0.0,
                                        op0=add, op1=amax)

            # ---- layer 3 + bias + store ----
            for m3 in range(3):
                ps3 = mps.tile([128, H3], F32, tag="mm")
                for kc in range(2):
                    nc.tensor.matmul(ps3, x2T[:, kc, m3 * 128:(m3 + 1) * 128],
                                     w3_sb[:, kc, :], start=(kc == 0), stop=(kc == 1))
                o_sb = tmpp.tile([128, H3], F32, tag="osb")
                nc.vector.tensor_tensor(o_sb, ps3, b3_sb, op=add)
                nc.sync.dma_start(out=out[m3 * 128:(m3 + 1) * 128, :], in_=o_sb)
    nc.compile()
    return nc


def _get_program():
    global _PROGRAM
    if _PROGRAM is None:
        _PROGRAM = _build_program()
    return _PROGRAM


def _preprocess(boxes, scores):
    """Rasterize boxes to 0/1 masks with detection columns in sorted order."""
    cx, cy, bw, bh = boxes[..., 0], boxes[..., 1], boxes[..., 2], boxes[..., 3]
    x1 = np.floor((cx - bw / 2) * GRID).astype(np.int64)
    y1 = np.floor((cy - bh / 2) * GRID).astype(np.int64)
    x2 = np.floor((cx + bw / 2) * GRID).astype(np.int64)
    y2 = np.floor((cy + bh / 2) * GRID).astype(np.int64)
    hidx = np.argsort(-scores[:, :NH], axis=1, kind="stable")
    oidx = np.argsort(-scores[:, NH:], axis=1, kind="stable") + NH
    perm = np.concatenate([hidx, oidx], axis=1)                     # [B, D]
    g = np.arange(GRID)
    rows = (g[None, None, :] >= y1[..., None]) & (g[None, None, :] < y2[..., None])
    cols = (g[None, None, :] >= x1[..., None]) & (g[None, None, :] < x2[..., None])
    rows = np.take_along_axis(rows, perm[..., None], axis=1)        # [B, D, 64]
    cols = np.take_along_axis(cols, perm[..., None], axis=1)
    area = rows.sum(-1) * cols.sum(-1)                              # [B, D]
    mask = rows[:, :, :, None] & cols[:, :, None, :]                # [B, D, 64, 64]
    maskT = np.ascontiguousarray(
        mask.reshape(mask.shape[0], D, KPIX).transpose(0, 2, 1)).astype(BF)
    return maskT, (1.0 / area).astype(np.float32)


def _run(in_maps, trace=False, **kw):
    nc = _get_program()
    return run_bass_kernel_spmd(nc, in_maps, core_ids=list(range(N_CORES)),
                                trace=trace, **kw)


def _make_in_maps(features, boxes, scores, w1, b1, w2, b2, w3, b3):
    features = np.asarray(features, np.float32)
    maskT, inva = _preprocess(np.asarray(boxes, np.float32),
                              np.asarray(scores, np.float32))
    featb = np.ascontiguousarray(features.reshape(B, KPIX, C)).astype(BF)
    w1b = np.asarray(w1, np.float32).astype(BF)
    w2b = np.asarray(w2, np.float32).astype(BF)
    w3b = np.asarray(w3, np.float32).astype(BF)
    b1f = np.asarray(b1, np.float32)
    b2f = np.asarray(b2, np.float32)
    b3f = np.asarray(b3, np.float32)
    in_maps = []
    for c in range(N_CORES):
        s = slice(c * BL, (c + 1) * BL)
        in_maps.append({
            "feat": np.ascontiguousarray(featb[s]),
            "maskT": np.ascontiguousarray(maskT[s]),
            "inva": np.ascontiguousarray(inva[s]),
            "w1": w1b, "b1": b1f, "w2": w2b, "b2": b2f, "w3": w3b, "b3": b3f,
        })
    return in_maps


def kernel(features, boxes, scores, w1, b1, w2, b2, w3, b3, labels):
    in_maps = _make_in_maps(features, boxes, scores, w1, b1, w2, b2, w3, b3)
    res = _run(in_maps, trace=False)
    out = np.concatenate([r["out"].reshape(BL, NPAIR, H3) for r in res.results],
                         axis=0)
    return np.ascontiguousarray(out.astype(np.float32))



# revision 2
# speedup vs baseline: 1.9204x; 1.9204x over previous
"""Trainium2 Bass kernel for the HOI relation model.

Per core (2 images, 8 cores data-parallel over batch):
  1. ROI mean pooling as K-chunk matmuls over a *packed* pixel list:
     only pixels inside the union of the 32 boxes are shipped/streamed
     (~44% of the 64x64 grid).  Features and masks travel as fp8 e3m4
     (4 mantissa bits), halving DMA bytes vs bf16; PSUM accumulates f32.
  2. PE-transpose pooled [32,768] -> pooledT [768,*] with persons and
     objects separated (persons cols 0:16, objects 16:64 across 2 imgs).
  3. Layer 1 factorized: relu(pair(h,o) @ w1 + b1) = relu(A(h)+B(o)+b1),
     A = w1[:768].T @ h (16 cols), B = w1[768:].T @ o (48 cols); the
     8x24 pair expansion happens after the matmul on DVE.
  4. Layers 2, 3 as plain matmuls on the 384 pair rows.

All DRAM tensors are laid out partition-major on the host so every DMA
moves large contiguous per-partition lines (no strided descriptors).
Host does layout/packing prep only: box rasterization, score argsort
(baked into mask column order), union-pixel gather, dtype casts.
"""

import numpy as np
import ml_dtypes

import concourse.bass as bass
import concourse.mybir as mybir
import concourse.tile as tile
from concourse import bacc
from concourse.bass_utils import run_bass_kernel_spmd
from concourse.masks import make_identity

N_CORES = 8
B, D, C = 16, 32, 768
NH, NO = 8, 24
NPAIR = NH * NO              # 192 pairs per image
GRID = 64                    # feature grid (896 / 14)
BL = B // N_CORES            # 2 images per core
H1, H2, H3 = 512, 256, 117
M = BL * NPAIR               # 384 pair rows per core

F32 = mybir.dt.float32
BF16 = mybir.dt.bfloat16
FP8 = mybir.dt.float8e3
BF = ml_dtypes.bfloat16
F8 = ml_dtypes.float8_e3m4

_PROGRAMS = {}


def _build_program(nch):
    """nch: number of 128-pixel K-chunks per image (padded packed pixels)."""
    nc = bacc.Bacc("TRN2", target_bir_lowering=False, debug=False,
                   num_devices=N_CORES)
    feat = nc.declare_dram_parameter("feat", [128, BL, nch, C], FP8,
                                     isOutput=False)
    maskT = nc.declare_dram_parameter("maskT", [128, BL, nch, D], FP8,
                                      isOutput=False)
    inva = nc.declare_dram_parameter("inva", [BL, D], F32, isOutput=False)
    w1 = nc.declare_dram_parameter("w1", [128, 12, H1], BF16, isOutput=False)
    b1 = nc.declare_dram_parameter("b1", [128, 4], F32, isOutput=False)
    w2 = nc.declare_dram_parameter("w2", [128, 4, H2], BF16, isOutput=False)
    b2 = nc.declare_dram_parameter("b2", [128, 2], F32, isOutput=False)
    w3 = nc.declare_dram_parameter("w3", [128, 2, H3], BF16, isOutput=False)
    b3 = nc.declare_dram_parameter("b3", [128, H3], F32, isOutput=False)
    out = nc.declare_dram_parameter("out", [M, H3], F32, isOutput=True)

    add = mybir.AluOpType.add
    amax = mybir.AluOpType.max
    ha = nch - nch // 2          # first feat piece chunks (per image)

    with tile.TileContext(nc) as tc:
        with (
            tc.tile_pool(name="singles", bufs=1) as singles,
            tc.tile_pool(name="work", bufs=1) as work,
            tc.tile_pool(name="tmp", bufs=3) as tmpp,
            tc.tile_pool(name="pps", bufs=1, space="PSUM") as pps,
            tc.tile_pool(name="mps", bufs=4, space="PSUM") as mps,
        ):
            # ---- DMA queue: mask+inva first, then features, then MLP ----
            ident = singles.tile([32, 32], BF16, tag="ident")
            make_identity(nc, ident)
            m_sb = singles.tile([128, BL, nch, D], FP8, tag="mask")
            nc.sync.dma_start(out=m_sb, in_=maskT[:, :, :, :])
            inva_sb = singles.tile([D, BL], F32, tag="inva")
            nc.sync.dma_start(out=inva_sb, in_=inva[:, :].rearrange("b d -> d b"))
            f_sb = []
            for img in range(BL):
                fa = singles.tile([128, ha, C], FP8, tag=f"feat{img}a")
                nc.sync.dma_start(out=fa, in_=feat[:, img, 0:ha, :])
                fb = singles.tile([128, nch - ha, C], FP8, tag=f"feat{img}b")
                nc.sync.dma_start(out=fb, in_=feat[:, img, ha:nch, :])
                f_sb.append((fa, fb))
            w1_sb = singles.tile([128, 12, H1], BF16, tag="w1")
            nc.sync.dma_start(out=w1_sb, in_=w1[:, :, :])
            b1_sb = singles.tile([128, 4], F32, tag="b1")
            nc.sync.dma_start(out=b1_sb, in_=b1[:, :])
            w2_sb = singles.tile([128, 4, H2], BF16, tag="w2")
            nc.sync.dma_start(out=w2_sb, in_=w2[:, :, :])
            b2_sb = singles.tile([128, 2], F32, tag="b2")
            nc.sync.dma_start(out=b2_sb, in_=b2[:, :])
            w3_sb = singles.tile([128, 2, H3], BF16, tag="w3")
            nc.sync.dma_start(out=w3_sb, in_=w3[:, :, :])
            b3_sb = singles.tile([128, H3], F32, tag="b3")
            nc.sync.dma_start(out=b3_sb, in_=b3[:, :])

            # persistent activations
            # pooledT cols: [h img0 (8) | h img1 (8) | o img0 (24) | o img1 (24)]
            pooledT = work.tile([128, 6, 2 * D], BF16, tag="pooledT")
            x1T = work.tile([128, 4, M], BF16, tag="x1T")
            x2T = work.tile([128, 2, M], BF16, tag="x2T")

            # ---- pooling + transpose per image ----
            for img in range(BL):
                fa, fb = f_sb[img]
                ps_a = pps.tile([D, 384], F32, tag=f"pp{img}a")
                ps_b = pps.tile([D, 384], F32, tag=f"pp{img}b")
                for kk in range(nch):
                    fsrc = fa[:, kk, :] if kk < ha else fb[:, kk - ha, :]
                    nc.tensor.matmul(ps_a, m_sb[:, img, kk, :], fsrc[:, 0:384],
                                     start=(kk == 0), stop=(kk == nch - 1))
                    nc.tensor.matmul(ps_b, m_sb[:, img, kk, :], fsrc[:, 384:768],
                                     start=(kk == 0), stop=(kk == nch - 1))
                # scale by 1/area, cast to bf16
                pooled = tmpp.tile([D, C], BF16, tag="pooled")
                nc.vector.tensor_scalar_mul(pooled[:, 0:384], ps_a,
                                            inva_sb[:, img:img + 1])
                nc.vector.tensor_scalar_mul(pooled[:, 384:768], ps_b,
                                            inva_sb[:, img:img + 1])
                # transpose to [C, D] in 6 chunks of 128 channels
                for cc in range(6):
                    ps_t = mps.tile([128, D], BF16, tag="mm")
                    nc.tensor.transpose(ps_t, pooled[:, cc * 128:(cc + 1) * 128],
                                        ident)
                    nc.vector.tensor_copy(
                        pooledT[:, cc, img * NH:(img + 1) * NH], ps_t[:, 0:NH])
                    nc.vector.tensor_copy(
                        pooledT[:, cc, 2 * NH + img * NO:2 * NH + (img + 1) * NO],
                        ps_t[:, NH:D])

            # ---- layer 1 (factorized over pairs) ----
            for mc in range(4):
                ps_h = mps.tile([128, 2 * NH], F32, tag="mm")
                ps_o = mps.tile([128, 2 * NO], F32, tag="mm")
                for kc in range(6):
                    nc.tensor.matmul(ps_h, w1_sb[:, kc, mc * 128:(mc + 1) * 128],
                                     pooledT[:, kc, 0:2 * NH],
                                     start=(kc == 0), stop=(kc == 5))
                for kc in range(6):
                    nc.tensor.matmul(ps_o, w1_sb[:, 6 + kc, mc * 128:(mc + 1) * 128],
                                     pooledT[:, kc, 2 * NH:2 * D],
                                     start=(kc == 0), stop=(kc == 5))
                ab_sb = tmpp.tile([128, 2 * D], F32, tag="ab")
                nc.vector.tensor_copy(ab_sb[:, 0:2 * NH], ps_h)
                nc.vector.tensor_copy(ab_sb[:, 2 * NH:2 * D], ps_o)
                for img in range(BL):
                    pre = tmpp.tile([128, NH, NO], F32, tag="pre")
                    a_bc = ab_sb[:, img * NH:(img + 1) * NH][:, :, None] \
                        .broadcast_to([128, NH, NO])
                    b_bc = ab_sb[:, 2 * NH + img * NO:2 * NH + (img + 1) * NO] \
                        [:, None, :].broadcast_to([128, NH, NO])
                    # pre = (A + b1) + B
                    nc.vector.scalar_tensor_tensor(pre, a_bc, b1_sb[:, mc:mc + 1],
                                                   b_bc, op0=add, op1=add)
                    dst = x1T[:, mc, img * NPAIR:(img + 1) * NPAIR] \
                        .rearrange("p (i j) -> p i j", i=NH)
                    nc.vector.tensor_scalar_max(dst, pre, 0.0)

            # ---- layer 2 ----
            for m2 in range(2):
                ps2 = mps.tile([128, M], F32, tag="mm")
                for kc in range(4):
                    nc.tensor.matmul(ps2, w2_sb[:, kc, m2 * 128:(m2 + 1) * 128],
                                     x1T[:, kc, :], start=(kc == 0), stop=(kc == 3))
                nc.vector.tensor_scalar(x2T[:, m2, :], ps2, b2_sb[:, m2:m2 + 1],
                                        0.0, op0=add, op1=amax)

            # ---- layer 3 + bias + store ----
            for m3 in range(3):
                ps3 = mps.tile([128, H3], F32, tag="mm")
                for kc in range(2):
                    nc.tensor.matmul(ps3, x2T[:, kc, m3 * 128:(m3 + 1) * 128],
                                     w3_sb[:, kc, :], start=(kc == 0), stop=(kc == 1))
                o_sb = tmpp.tile([128, H3], F32, tag="osb")
                nc.vector.tensor_tensor(o_sb, ps3, b3_sb, op=add)
                nc.sync.dma_start(out=out[m3 * 128:(m3 + 1) * 128, :], in_=o_sb)
    nc.compile()
    return nc


def _get_program(nch):
    if nch not in _PROGRAMS:
        _PROGRAMS[nch] = _build_program(nch)
    return _PROGRAMS[nch]


def _preprocess(features, boxes, scores):
    """Pack union-of-boxes pixels; rasterize masks with detection columns in
    sorted-score order. Returns partition-major fp8 feat/mask + 1/area."""
    cx, cy, bw, bh = boxes[..., 0], boxes[..., 1], boxes[..., 2], boxes[..., 3]
    x1 = np.floor((cx - bw / 2) * GRID).astype(np.int64)
    y1 = np.floor((cy - bh / 2) * GRID).astype(np.int64)
    x2 = np.floor((cx + bw / 2) * GRID).astype(np.int64)
    y2 = np.floor((cy + bh / 2) * GRID).astype(np.int64)
    hidx = np.argsort(-scores[:, :NH], axis=1, kind="stable")
    oidx = np.argsort(-scores[:, NH:], axis=1, kind="stable") + NH
    perm = np.concatenate([hidx, oidx], axis=1)                     # [B, D]
    g = np.arange(GRID)
    rows = (g[None, None, :] >= y1[..., None]) & (g[None, None, :] < y2[..., None])
    cols = (g[None, None, :] >= x1[..., None]) & (g[None, None, :] < x2[..., None])
    rows = np.take_along_axis(rows, perm[..., None], axis=1)        # [B, D, 64]
    cols = np.take_along_axis(cols, perm[..., None], axis=1)
    area = rows.sum(-1) * cols.sum(-1)                              # [B, D]
    # union of all boxes per image; pack only covered pixels
    union = (rows[:, :, :, None] & cols[:, :, None, :]).any(axis=1)  # [B,64,64]
    uflat = union.reshape(B, GRID * GRID)
    counts = uflat.sum(1)
    nch = int(-(-counts.max() // 128))
    pp = nch * 128
    pix = np.zeros((B, pp), np.int64)
    valid = np.zeros((B, pp), bool)
    for b in range(B):
        idx = np.flatnonzero(uflat[b])
        pix[b, :len(idx)] = idx
        valid[b, :len(idx)] = True
    featP = features.reshape(B, GRID * GRID, C)[np.arange(B)[:, None], pix]
    featP[~valid] = 0.0
    py, px = pix // GRID, pix % GRID
    mrow = np.take_along_axis(rows, py[:, None, :], axis=2)          # [B, D, pp]
    mcol = np.take_along_axis(cols, px[:, None, :], axis=2)
    maskP = (mrow & mcol & valid[:, None, :]).transpose(0, 2, 1)     # [B, pp, D]
    return (featP.astype(F8), maskP.astype(F8),
            (1.0 / area).astype(np.float32), nch)


def _pmajor(a, nch):
    """[BL, nch*128, X] -> contiguous [128, BL, nch, X]."""
    bl, _, x = a.shape
    return np.ascontiguousarray(
        a.reshape(bl, nch, 128, x).transpose(2, 0, 1, 3))


def _run(in_maps, trace=False, **kw):
    nch = in_maps[0]["feat"].shape[2]
    nc = _get_program(nch)
    return run_bass_kernel_spmd(nc, in_maps, core_ids=list(range(N_CORES)),
                                trace=trace, **kw)


def _make_in_maps(features, boxes, scores, w1, b1, w2, b2, w3, b3):
    features = np.asarray(features, np.float32)
    featP, maskP, inva, nch = _preprocess(
        features, np.asarray(boxes, np.float32), np.asarray(scores, np.float32))
    w1b = np.ascontiguousarray(np.asarray(w1, np.float32).astype(BF)
                               .reshape(12, 128, H1).transpose(1, 0, 2))
    w2b = np.ascontiguousarray(np.asarray(w2, np.float32).astype(BF)
                               .reshape(4, 128, H2).transpose(1, 0, 2))
    w3b = np.ascontiguousarray(np.asarray(w3, np.float32).astype(BF)
                               .reshape(2, 128, H3).transpose(1, 0, 2))
    b1f = np.ascontiguousarray(np.asarray(b1, np.float32).reshape(4, 128).T)
    b2f = np.ascontiguousarray(np.asarray(b2, np.float32).reshape(2, 128).T)
    b3f = np.ascontiguousarray(
        np.broadcast_to(np.asarray(b3, np.float32), (128, H3)))
    in_maps = []
    for c in range(N_CORES):
        s = slice(c * BL, (c + 1) * BL)
        in_maps.append({
            "feat": _pmajor(featP[s], nch),
            "maskT": _pmajor(maskP[s], nch),
            "inva": np.ascontiguousarray(inva[s]),
            "w1": w1b, "b1": b1f, "w2": w2b, "b2": b2f, "w3": w3b, "b3": b3f,
        })
    return in_maps


def kernel(features, boxes, scores, w1, b1, w2, b2, w3, b3, labels):
    in_maps = _make_in_maps(features, boxes, scores, w1, b1, w2, b2, w3, b3)
    res = _run(in_maps, trace=False)
    out = np.concatenate([r["out"].reshape(BL, NPAIR, H3) for r in res.results],
                         axis=0)
    return np.ascontiguousarray(out.astype(np.float32))
